# revision 14
# baseline (speedup 1.0000x reference)
"""GCN (gather-scale-segment_max x2) on 8 Trainium2 NeuronCores.

Strategy (inspector-executor, 2 SPMD launches):
  Edges are sharded by destination-node block (12500 nodes per core), so each
  core owns the complete reduction for its nodes and no cross-core reduce is
  needed.  The host (inspector) builds dst-grouped slot tables -- for every
  node a padded set of 16-slot rows listing its in-edges -- and materialises
  the per-slot operand tables msg = x[src[slot]] / ts[slot] by pure index
  operations (the same index-only role the host played in the previous
  4-launch version, which shipped host-permuted per-edge message tables into
  its reduce launches).  All arithmetic runs on the NeuronCores (executor):

    launch 1: msg*ts -> two-level segmented max -> @W1+b1 -> relu    [per core]
    launch 2: (h gathered to slots on host) msg*ts -> segmented max -> @W2+b2

  Device-side structure per launch (all engines overlap):
    - stream 512-slot stripes [128, 512, F]: DMA in (msg tables in bf16 to
      halve HBM traffic; ts and all accumulation stay f32 -- end-to-end
      rel err ~6.5e-3 vs the 2e-2 gate), DVE multiply by ts,
      DVE 16-slot segmented max -> part rows
    - DVE second-level max over each node's rows -> agg [128, M, F]
    - batched tail: 8 (16) node-groups per PE transpose -> one block-diagonal
      matmul [128x64] ([128x32]) -> ACT bias(+relu) -> DMA out
"""

import os

os.environ.setdefault("JAX_COMPILATION_CACHE_DIR", "/tmp/jax_kernel_cache")

import numpy as np
from concourse import bass, mybir
from concourse.bass_utils import run_bass_kernel_spmd
from concourse.tile import TileContext
from bass_rust import ScopedClock

try:
    import jax
    jax.config.update("jax_compilation_cache_dir", "/tmp/jax_kernel_cache")
    jax.config.update("jax_persistent_cache_min_compile_time_secs", 0.5)
except Exception:
    pass

N_CORES = 8
N_NODES = 100000
B = N_NODES // N_CORES          # 12500 dst nodes per core
P = 128
F1, HID, NCLS = 16, 8, 2
STRIPE_ROWS = 32                 # 32 rows x 16 slots = 512 slots per stripe
G1 = 8                           # node-groups per matmul, layer 1 (8*16=128)
G2 = 16                          # node-groups per matmul, layer 2 (16*8=128)

_DT = mybir.dt.float32
_BF = mybir.dt.bfloat16


# ---------------------------------------------------------------- tile patch
class _Tc(TileContext):
    """This walrus build allows only ONE sync-wait per instruction; split the
    end-of-kernel drain waits across SP nops."""

    def _drain_and_barrier(self, tick_clock, wait_clock):
        holder = self.nc.sync.nop(nofuse=True, hint="drain_waits")
        wait_clock.add_sem_waits(holder.ins, ScopedClock({None: tick_clock.global_clock}))
        si = holder.ins.sync_info
        waits = list(si.on_wait) if si and si.on_wait else []
        if len(waits) > 1:
            upd = list(si.on_update) if si.on_update else []
            holder.ins.sync_info = mybir.SyncInfo(on_wait=waits[:1], on_update=upd)
            for w in waits[1:]:
                extra = self.nc.sync.nop(nofuse=True, hint="drain_waits")
                extra.ins.sync_info = mybir.SyncInfo(on_wait=[w], on_update=[])
        self.nc.sync.drain()
        self.nc.all_engine_barrier()
        assert self.sems is not None
        popped = self.nc._tile_sem_poison_stack.pop()
        assert popped is self._sem_poison
        self.nc.clear_and_free_semaphores(list(self.sems.allocated().values()))
        self.nc.all_engine_barrier()


def _split_waits(nc, max_waits=1):
    n = 0
    for fn in nc.m.functions:
        for bb in fn.blocks:
            out = []
            for inst in bb.instructions:
                si = inst.sync_info
                waits = list(si.on_wait) if si and si.on_wait else []
                if len(waits) > max_waits:
                    for w in waits[:-max_waits]:
                        n += 1
                        nop = mybir.InstNoOp(name=f"I-ws-{n}")
                        nop.engine = inst.engine
                        nop.sync_info = mybir.SyncInfo(on_wait=[w], on_update=[])
                        out.append(nop)
                    inst.sync_info = mybir.SyncInfo(
                        on_wait=waits[-max_waits:],
                        on_update=list(si.on_update) if si.on_update else [],
                    )
                out.append(inst)
            bb.instructions[:] = out
    return n


# ------------------------------------------------------------- host indexing
class _Shard:
    pass


def _prepare(src, dst, ts):
    """Per-core shards plus cross-core-uniform slot-table structure."""
    shards = []
    blk = dst // B
    for n in range(N_CORES):
        sh = _Shard()
        sel = np.nonzero(blk == n)[0]
        sh.es = src[sel].astype(np.int64)
        sh.ts = ts[sel]
        ed = (dst[sel] - n * B).astype(np.int64)
        sh.order_d = np.argsort(ed, kind="stable").astype(np.int64)
        sh.deg = np.bincount(ed, minlength=B).astype(np.int64)
        assert sh.deg.min() >= 1
        sh.dstart = np.concatenate([[0], np.cumsum(sh.deg)[:-1]])
        sh.rows_i = -(-sh.deg // 16)
        shards.append(sh)

    # uniform rowcount-group structure (same grid shape on every core)
    rmax = max(int(sh.rows_i.max()) for sh in shards)
    count_r = np.zeros(rmax + 1, np.int64)
    for sh in shards:
        count_r = np.maximum(count_r, np.bincount(sh.rows_i, minlength=rmax + 1))
    count_r[0] = 0
    m_r = -(-count_r // P)          # node-grid rows of 128 nodes, per rowcount
    m_r[1] += (-int(m_r.sum())) % G2     # M multiple of 16 for matmul batching
    M = int(m_r.sum())
    rows_pp = int((m_r * np.arange(rmax + 1)).sum())
    ROWS_PAD = -(-rows_pp // STRIPE_ROWS) * STRIPE_ROWS
    S_PP = ROWS_PAD * 16
    r_list = [int(r) for r in np.nonzero(m_r)[0]]
    groups = [(r, int(m_r[r])) for r in r_list]

    for sh in shards:
        grids = []
        slot_chunks = []
        for r in r_list:
            nodes = np.nonzero(sh.rows_i == r)[0]
            need = int(m_r[r]) * P
            g = np.full(need, -1, np.int64)
            g[:len(nodes)] = nodes
            g = g.reshape(int(m_r[r]), P)
            grids.append(g)
            gg = np.where(g < 0, 0, g)
            start = sh.dstart[gg][:, :, None]
            degg = sh.deg[gg][:, :, None]
            j = np.arange(16 * r)[None, None, :]
            eidx = sh.order_d[start + np.minimum(j, degg - 1)]
            slot_chunks.append(eidx.transpose(1, 0, 2).reshape(P, -1))
        sh.node_grid = np.concatenate(grids, axis=0)          # [M, P]
        se = np.concatenate(slot_chunks, axis=1)              # [P, rows_pp*16]
        if se.shape[1] < S_PP:                                # stripe padding
            pad = np.zeros((P, S_PP - se.shape[1]), np.int64)
            se = np.concatenate([se, pad], axis=1)
        sh.slot_edge = se
        assert sh.slot_edge.shape == (P, S_PP)

    cfg = dict(GROUPS=groups, ROWS_PAD=ROWS_PAD, S_PP=S_PP, M=M)
    return shards, cfg


def _blockdiag(W, G):
    f, h = W.shape
    wb = np.zeros((G * f, G * h), np.float32)
    for j in range(G):
        wb[j * f:(j + 1) * f, j * h:(j + 1) * h] = W
    return wb


# ------------------------------------------------------------ device builds
def _build_reduce(cfg, feat, hidden, G, relu, reps=1):
    """msg*ts -> two-level segmented max -> batched (@Wblk + b) [-> relu]."""
    S_PP, ROWS_PAD, M = cfg["S_PP"], cfg["ROWS_PAD"], cfg["M"]
    n_stripes = ROWS_PAD // STRIPE_ROWS
    SS = STRIPE_ROWS * 16
    MB = M // G
    assert G * feat == P

    nc = bass.Bass("TRN2", target_bir_lowering=False, debug=False,
                   num_devices=N_CORES)
    msg = nc.declare_dram_parameter("msg", [P, ROWS_PAD, feat, 16], _BF,
                                    isOutput=False)
    tsd = nc.declare_dram_parameter("ts", [P, ROWS_PAD, 16], _DT, isOutput=False)
    wd = nc.declare_dram_parameter("w", [G * feat, G * hidden], _DT, isOutput=False)
    bd = nc.declare_dram_parameter("b", [G * hidden, 1], _DT, isOutput=False)
    ident = nc.declare_dram_parameter("ident", [P, P], _DT, isOutput=False)
    hT = nc.declare_dram_parameter("hT", [MB, G * hidden, P], _DT, isOutput=True)

    with _Tc(nc) as tc:
        with tc.tile_pool(name="sb", bufs=2) as sb, \
             tc.tile_pool(name="big", bufs=1) as big, \
             tc.tile_pool(name="cst", bufs=1) as cst, \
             tc.tile_pool(name="ps", bufs=4, space="PSUM") as ps:
            w_t = cst.tile([G * feat, G * hidden], _DT)
            nc.sync.dma_start(out=w_t[:, :], in_=wd[:, :])
            b_t = cst.tile([G * hidden, 1], _DT)
            nc.sync.dma_start(out=b_t[:, :], in_=bd[:, :])
            id_t = cst.tile([P, P], _DT)
            nc.sync.dma_start(out=id_t[:, :], in_=ident[:, :])

            part = big.tile([P, ROWS_PAD, feat], _DT)
            agg = big.tile([P, M, feat], _DT)
            func = (mybir.ActivationFunctionType.Relu if relu
                    else mybir.ActivationFunctionType.Identity)

            for _rep in range(reps):
                for st in range(n_stripes):
                    r0, r1 = st * STRIPE_ROWS, (st + 1) * STRIPE_ROWS
                    mt = sb.tile([P, STRIPE_ROWS, feat, 16], _BF, tag="mt")
                    nc.sync.dma_start(out=mt[:, :, :, :],
                                      in_=msg[:, r0:r1, :, :])
                    tt = sb.tile([P, STRIPE_ROWS, 16], _DT, tag="tt")
                    nc.scalar.dma_start(out=tt[:, :, :],
                                        in_=tsd[:, r0:r1, :])
                    mf = sb.tile([P, STRIPE_ROWS, feat, 16], _DT, tag="mf")
                    nc.vector.tensor_tensor(
                        out=mf[:, :, :, :], in0=mt[:, :, :, :],
                        in1=tt[:, :, None, :].to_broadcast(
                            [P, STRIPE_ROWS, feat, 16]),
                        op=mybir.AluOpType.mult)
                    nc.vector.tensor_reduce(
                        out=part[:, r0:r1, :],
                        in_=mf[:, :, :, :],
                        axis=mybir.AxisListType.X, op=mybir.AluOpType.max)

                row0 = node0 = 0
                for r, m in cfg["GROUPS"]:
                    nc.vector.tensor_reduce(
                        out=agg[:, node0:node0 + m, :],
                        in_=part[:, row0:row0 + r * m, :].rearrange(
                            "p (m r) f -> p m f r", r=r),
                        axis=mybir.AxisListType.X, op=mybir.AluOpType.max)
                    row0 += r * m
                    node0 += m

                for mb in range(MB):
                    atp = ps.tile([P, P], _DT, tag="tp")
                    nc.tensor.transpose(
                        out=atp[:, :],
                        in_=agg[:, mb * G:(mb + 1) * G, :].rearrange(
                            "p m f -> p (m f)"),
                        identity=id_t[:, :])
                    ats = sb.tile([P, P], _DT, tag="ats")
                    nc.scalar.copy(out=ats[:, :], in_=atp[:, :])
                    hp = ps.tile([G * hidden, P], _DT, tag="hp")
                    nc.tensor.matmul(out=hp[:, :], lhsT=w_t[:, :], rhs=ats[:, :],
                                     start=True, stop=True)
                    hs = sb.tile([G * hidden, P], _DT, tag="hs")
                    nc.scalar.activation(out=hs[:, :], in_=hp[:, :], func=func,
                                         bias=b_t[:, :], scale=1.0)
                    nc.sync.dma_start(out=hT[mb, :, :], in_=hs[:, :])
    _split_waits(nc)
    return nc


# ------------------------------------------------------------------- kernel
LAST_TIMINGS = {}


def _in_maps(shards, x_or_h, W, bvec, G, feat):
    import ml_dtypes
    wblk = _blockdiag(np.asarray(W, np.float32), G)
    bblk = np.ascontiguousarray(np.tile(np.asarray(bvec, np.float32), G)[:, None])
    identv = np.eye(P, dtype=np.float32)
    maps = []
    for sh in shards:
        S_PP = sh.slot_edge.shape[1]
        rows = x_or_h[sh.es[sh.slot_edge]]             # [P, S_PP, feat]
        rows = rows.reshape(P, S_PP // 16, 16, feat).transpose(0, 1, 3, 2)
        tss = sh.ts[sh.slot_edge].reshape(P, S_PP // 16, 16)
        maps.append({
            "msg": np.ascontiguousarray(rows, dtype=ml_dtypes.bfloat16),
            "ts": np.ascontiguousarray(tss, dtype=np.float32),
            "w": wblk, "b": bblk, "ident": identv,
        })
    return maps


def _unshard(shards, cfg, rT_list, hid, G):
    """[MB, G*hid, P] per core -> full [N_NODES, hid]."""
    M = cfg["M"]
    out = np.zeros((N_NODES, hid), np.float32)
    for n, sh in enumerate(shards):
        rT = rT_list[n]                                # [MB, G*hid, P]
        hb = rT.reshape(M // G, G, hid, P).transpose(0, 1, 3, 2)
        hb = hb.reshape(M, P, hid)                     # grid-ordered rows
        valid = sh.node_grid >= 0
        out[n * B + sh.node_grid[valid]] = hb[valid]
    return out


def kernel(x, src, dst, timestamp, W1, b1, W2, b2):
    import time as _time
    x = np.ascontiguousarray(np.asarray(x, np.float32))
    src = np.asarray(src, np.int32)
    dst = np.asarray(dst, np.int32)
    timestamp = np.asarray(timestamp, np.float32)

    shards, cfg = _prepare(src, dst, timestamp)
    cores = list(range(N_CORES))

    # ---- launch 1: scale + segment max + linear1 + relu
    nc1 = _build_reduce(cfg, F1, HID, G1, relu=True)
    in1 = _in_maps(shards, x, W1, b1, G1, F1)
    _t = _time.time()
    r1 = run_bass_kernel_spmd(nc1, in1, cores).results
    LAST_TIMINGS["reduce_1"] = _time.time() - _t

    h_full = _unshard(shards, cfg, [r["hT"] for r in r1], HID, G1)

    # ---- launch 2: scale + segment max + linear2
    nc2 = _build_reduce(cfg, HID, NCLS, G2, relu=False)
    in2 = _in_maps(shards, h_full, W2, b2, G2, HID)
    _t = _time.time()
    r2 = run_bass_kernel_spmd(nc2, in2, cores).results
    LAST_TIMINGS["reduce_2"] = _time.time() - _t

    return _unshard(shards, cfg, [r["hT"] for r in r2], NCLS, G2)


# revision 20
# speedup vs baseline: 1.0536x; 1.0536x over previous
"""GCN (gather-scale-segment_max x2) on 8 Trainium2 NeuronCores.

Strategy (inspector-executor, 2 SPMD launches):
  Edges are sharded by destination-node block (12500 nodes per core), so each
  core owns the complete reduction for its nodes and no cross-core reduce is
  needed.  The host (inspector) builds dst-grouped slot tables -- for every
  node a padded set of 16-slot rows listing its in-edges -- and materialises
  the per-slot operand tables msg = x[src[slot]] / ts[slot] by pure index
  operations (the same index-only role the host played in the previous
  4-launch version, which shipped host-permuted per-edge message tables into
  its reduce launches).  All arithmetic runs on the NeuronCores (executor):

    launch 1: msg*ts -> two-level segmented max -> @W1+b1 -> relu    [per core]
    launch 2: (h gathered to slots on host) msg*ts -> segmented max -> @W2+b2

  Device-side structure per launch (all engines overlap):
    - stream 512-slot stripes [128, 512, F]: DMA in (msg/ts tables and the
      whole segmented max in bf16 -- halves HBM traffic and engages the DVE
      16-bit 2x path; linear layers accumulate in f32 PSUM; end-to-end
      rel err ~1.15e-2 vs the 2e-2 gate), DVE in-place multiply by ts,
      DVE 16-slot segmented max -> part rows
    - DVE second-level max over each node's rows -> agg [128, M, F]
    - batched tail: 8 (16) node-groups per PE transpose -> one block-diagonal
      matmul [128x64] ([128x32]) -> ACT bias(+relu) -> DMA out
"""

import os

os.environ.setdefault("JAX_COMPILATION_CACHE_DIR", "/tmp/jax_kernel_cache")

import numpy as np
from concourse import bass, mybir
from concourse.bass_utils import run_bass_kernel_spmd
from concourse.tile import TileContext
from bass_rust import ScopedClock

try:
    import jax
    jax.config.update("jax_compilation_cache_dir", "/tmp/jax_kernel_cache")
    jax.config.update("jax_persistent_cache_min_compile_time_secs", 0.5)
except Exception:
    pass

N_CORES = 8
N_NODES = 100000
B = N_NODES // N_CORES          # 12500 dst nodes per core
P = 128
F1, HID, NCLS = 16, 8, 2
STRIPE_ROWS = 32                 # 32 rows x 16 slots = 512 slots per stripe
G1 = 8                           # node-groups per matmul, layer 1 (8*16=128)
G2 = 16                          # node-groups per matmul, layer 2 (16*8=128)

_DT = mybir.dt.float32
_BF = mybir.dt.bfloat16


# ---------------------------------------------------------------- tile patch
class _Tc(TileContext):
    """This walrus build allows only ONE sync-wait per instruction; split the
    end-of-kernel drain waits across SP nops."""

    def _drain_and_barrier(self, tick_clock, wait_clock):
        holder = self.nc.sync.nop(nofuse=True, hint="drain_waits")
        wait_clock.add_sem_waits(holder.ins, ScopedClock({None: tick_clock.global_clock}))
        si = holder.ins.sync_info
        waits = list(si.on_wait) if si and si.on_wait else []
        if len(waits) > 1:
            upd = list(si.on_update) if si.on_update else []
            holder.ins.sync_info = mybir.SyncInfo(on_wait=waits[:1], on_update=upd)
            for w in waits[1:]:
                extra = self.nc.sync.nop(nofuse=True, hint="drain_waits")
                extra.ins.sync_info = mybir.SyncInfo(on_wait=[w], on_update=[])
        self.nc.sync.drain()
        self.nc.all_engine_barrier()
        assert self.sems is not None
        popped = self.nc._tile_sem_poison_stack.pop()
        assert popped is self._sem_poison
        self.nc.clear_and_free_semaphores(list(self.sems.allocated().values()))
        self.nc.all_engine_barrier()


def _split_waits(nc, max_waits=1):
    n = 0
    for fn in nc.m.functions:
        for bb in fn.blocks:
            out = []
            for inst in bb.instructions:
                si = inst.sync_info
                waits = list(si.on_wait) if si and si.on_wait else []
                if len(waits) > max_waits:
                    for w in waits[:-max_waits]:
                        n += 1
                        nop = mybir.InstNoOp(name=f"I-ws-{n}")
                        nop.engine = inst.engine
                        nop.sync_info = mybir.SyncInfo(on_wait=[w], on_update=[])
                        out.append(nop)
                    inst.sync_info = mybir.SyncInfo(
                        on_wait=waits[-max_waits:],
                        on_update=list(si.on_update) if si.on_update else [],
                    )
                out.append(inst)
            bb.instructions[:] = out
    return n


# ------------------------------------------------------------- host indexing
class _Shard:
    pass


def _prepare(src, dst, ts):
    """Per-core shards plus cross-core-uniform slot-table structure."""
    shards = []
    blk = dst // B
    for n in range(N_CORES):
        sh = _Shard()
        sel = np.nonzero(blk == n)[0]
        sh.es = src[sel].astype(np.int64)
        sh.ts = ts[sel]
        ed = (dst[sel] - n * B).astype(np.int64)
        sh.order_d = np.argsort(ed, kind="stable").astype(np.int64)
        sh.deg = np.bincount(ed, minlength=B).astype(np.int64)
        assert sh.deg.min() >= 1
        sh.dstart = np.concatenate([[0], np.cumsum(sh.deg)[:-1]])
        sh.rows_i = -(-sh.deg // 16)
        shards.append(sh)

    # uniform rowcount-group structure (same grid shape on every core)
    rmax = max(int(sh.rows_i.max()) for sh in shards)
    count_r = np.zeros(rmax + 1, np.int64)
    for sh in shards:
        count_r = np.maximum(count_r, np.bincount(sh.rows_i, minlength=rmax + 1))
    count_r[0] = 0
    m_r = -(-count_r // P)          # node-grid rows of 128 nodes, per rowcount
    m_r[1] += (-int(m_r.sum())) % G2     # M multiple of 16 for matmul batching
    M = int(m_r.sum())
    rows_pp = int((m_r * np.arange(rmax + 1)).sum())
    ROWS_PAD = -(-rows_pp // STRIPE_ROWS) * STRIPE_ROWS
    S_PP = ROWS_PAD * 16
    r_list = [int(r) for r in np.nonzero(m_r)[0]]
    groups = [(r, int(m_r[r])) for r in r_list]

    for sh in shards:
        grids = []
        slot_chunks = []
        for r in r_list:
            nodes = np.nonzero(sh.rows_i == r)[0]
            need = int(m_r[r]) * P
            g = np.full(need, -1, np.int64)
            g[:len(nodes)] = nodes
            g = g.reshape(int(m_r[r]), P)
            grids.append(g)
            gg = np.where(g < 0, 0, g)
            start = sh.dstart[gg][:, :, None]
            degg = sh.deg[gg][:, :, None]
            j = np.arange(16 * r)[None, None, :]
            eidx = sh.order_d[start + np.minimum(j, degg - 1)]
            slot_chunks.append(eidx.transpose(1, 0, 2).reshape(P, -1))
        sh.node_grid = np.concatenate(grids, axis=0)          # [M, P]
        se = np.concatenate(slot_chunks, axis=1)              # [P, rows_pp*16]
        if se.shape[1] < S_PP:                                # stripe padding
            pad = np.zeros((P, S_PP - se.shape[1]), np.int64)
            se = np.concatenate([se, pad], axis=1)
        sh.slot_edge = se
        assert sh.slot_edge.shape == (P, S_PP)

    cfg = dict(GROUPS=groups, ROWS_PAD=ROWS_PAD, S_PP=S_PP, M=M)
    return shards, cfg


def _blockdiag(W, G):
    f, h = W.shape
    wb = np.zeros((G * f, G * h), np.float32)
    for j in range(G):
        wb[j * f:(j + 1) * f, j * h:(j + 1) * h] = W
    return wb


# ------------------------------------------------------------ device builds
def _build_reduce(cfg, feat, hidden, G, relu, reps=1):
    """msg*ts -> two-level segmented max -> batched (@Wblk + b) [-> relu]."""
    S_PP, ROWS_PAD, M = cfg["S_PP"], cfg["ROWS_PAD"], cfg["M"]
    n_stripes = ROWS_PAD // STRIPE_ROWS
    SS = STRIPE_ROWS * 16
    MB = M // G
    assert G * feat == P

    nc = bass.Bass("TRN2", target_bir_lowering=False, debug=False,
                   num_devices=N_CORES)
    msg = nc.declare_dram_parameter("msg", [P, ROWS_PAD, feat, 16], _BF,
                                    isOutput=False)
    tsd = nc.declare_dram_parameter("ts", [P, ROWS_PAD, 16], _BF, isOutput=False)
    wd = nc.declare_dram_parameter("w", [G * feat, G * hidden], _DT, isOutput=False)
    bd = nc.declare_dram_parameter("b", [G * hidden, 1], _DT, isOutput=False)
    ident = nc.declare_dram_parameter("ident", [P, P], _BF, isOutput=False)
    hT = nc.declare_dram_parameter("hT", [MB, G * hidden, P], _DT, isOutput=True)

    with _Tc(nc) as tc:
        with tc.tile_pool(name="sb", bufs=2) as sb, \
             tc.tile_pool(name="big", bufs=1) as big, \
             tc.tile_pool(name="cst", bufs=1) as cst, \
             tc.tile_pool(name="ps", bufs=4, space="PSUM") as ps:
            w_t = cst.tile([G * feat, G * hidden], _DT)
            nc.sync.dma_start(out=w_t[:, :], in_=wd[:, :])
            b_t = cst.tile([G * hidden, 1], _DT)
            nc.sync.dma_start(out=b_t[:, :], in_=bd[:, :])
            id_t = cst.tile([P, P], _BF)
            nc.sync.dma_start(out=id_t[:, :], in_=ident[:, :])

            part = big.tile([P, ROWS_PAD, feat], _BF)
            agg = big.tile([P, M, feat], _BF)
            func = (mybir.ActivationFunctionType.Relu if relu
                    else mybir.ActivationFunctionType.Identity)

            for _rep in range(reps):
                for st in range(n_stripes):
                    r0, r1 = st * STRIPE_ROWS, (st + 1) * STRIPE_ROWS
                    mt = sb.tile([P, STRIPE_ROWS, feat, 16], _BF, tag="mt")
                    nc.sync.dma_start(out=mt[:, :, :, :],
                                      in_=msg[:, r0:r1, :, :])
                    tt = sb.tile([P, STRIPE_ROWS, 16], _BF, tag="tt")
                    nc.scalar.dma_start(out=tt[:, :, :],
                                        in_=tsd[:, r0:r1, :])
                    nc.vector.tensor_tensor(
                        out=mt[:, :, :, :], in0=mt[:, :, :, :],
                        in1=tt[:, :, None, :].to_broadcast(
                            [P, STRIPE_ROWS, feat, 16]),
                        op=mybir.AluOpType.mult)
                    nc.vector.tensor_reduce(
                        out=part[:, r0:r1, :],
                        in_=mt[:, :, :, :],
                        axis=mybir.AxisListType.X, op=mybir.AluOpType.max)

                row0 = node0 = 0
                for r, m in cfg["GROUPS"]:
                    nc.vector.tensor_reduce(
                        out=agg[:, node0:node0 + m, :],
                        in_=part[:, row0:row0 + r * m, :].rearrange(
                            "p (m r) f -> p m f r", r=r),
                        axis=mybir.AxisListType.X, op=mybir.AluOpType.max)
                    row0 += r * m
                    node0 += m

                for mb in range(MB):
                    atp = ps.tile([P, P], _BF, tag="tp")
                    nc.tensor.transpose(
                        out=atp[:, :],
                        in_=agg[:, mb * G:(mb + 1) * G, :].rearrange(
                            "p m f -> p (m f)"),
                        identity=id_t[:, :])
                    ats = sb.tile([P, P], _DT, tag="ats")
                    nc.scalar.copy(out=ats[:, :], in_=atp[:, :])
                    hp = ps.tile([G * hidden, P], _DT, tag="hp")
                    nc.tensor.matmul(out=hp[:, :], lhsT=w_t[:, :], rhs=ats[:, :],
                                     start=True, stop=True)
                    hs = sb.tile([G * hidden, P], _DT, tag="hs")
                    nc.scalar.activation(out=hs[:, :], in_=hp[:, :], func=func,
                                         bias=b_t[:, :], scale=1.0)
                    nc.sync.dma_start(out=hT[mb, :, :], in_=hs[:, :])
    _split_waits(nc)
    return nc


# ------------------------------------------------------------------- kernel
LAST_TIMINGS = {}


def _in_maps(shards, x_or_h, W, bvec, G, feat):
    import ml_dtypes
    wblk = _blockdiag(np.asarray(W, np.float32), G)
    bblk = np.ascontiguousarray(np.tile(np.asarray(bvec, np.float32), G)[:, None])
    identv = np.eye(P, dtype=np.float32)
    maps = []
    for sh in shards:
        S_PP = sh.slot_edge.shape[1]
        rows = x_or_h[sh.es[sh.slot_edge]]             # [P, S_PP, feat]
        rows = rows.reshape(P, S_PP // 16, 16, feat).transpose(0, 1, 3, 2)
        tss = sh.ts[sh.slot_edge].reshape(P, S_PP // 16, 16)
        maps.append({
            "msg": np.ascontiguousarray(rows, dtype=ml_dtypes.bfloat16),
            "ts": np.ascontiguousarray(tss, dtype=ml_dtypes.bfloat16),
            "w": wblk, "b": bblk,
            "ident": identv.astype(ml_dtypes.bfloat16),
        })
    return maps


def _unshard(shards, cfg, rT_list, hid, G):
    """[MB, G*hid, P] per core -> full [N_NODES, hid]."""
    M = cfg["M"]
    out = np.zeros((N_NODES, hid), np.float32)
    for n, sh in enumerate(shards):
        rT = rT_list[n]                                # [MB, G*hid, P]
        hb = rT.reshape(M // G, G, hid, P).transpose(0, 1, 3, 2)
        hb = hb.reshape(M, P, hid)                     # grid-ordered rows
        valid = sh.node_grid >= 0
        out[n * B + sh.node_grid[valid]] = hb[valid]
    return out


def kernel(x, src, dst, timestamp, W1, b1, W2, b2):
    import time as _time
    x = np.ascontiguousarray(np.asarray(x, np.float32))
    src = np.asarray(src, np.int32)
    dst = np.asarray(dst, np.int32)
    timestamp = np.asarray(timestamp, np.float32)

    shards, cfg = _prepare(src, dst, timestamp)
    cores = list(range(N_CORES))

    # ---- launch 1: scale + segment max + linear1 + relu
    nc1 = _build_reduce(cfg, F1, HID, G1, relu=True)
    in1 = _in_maps(shards, x, W1, b1, G1, F1)
    _t = _time.time()
    r1 = run_bass_kernel_spmd(nc1, in1, cores).results
    LAST_TIMINGS["reduce_1"] = _time.time() - _t

    h_full = _unshard(shards, cfg, [r["hT"] for r in r1], HID, G1)

    # ---- launch 2: scale + segment max + linear2
    nc2 = _build_reduce(cfg, HID, NCLS, G2, relu=False)
    in2 = _in_maps(shards, h_full, W2, b2, G2, HID)
    _t = _time.time()
    r2 = run_bass_kernel_spmd(nc2, in2, cores).results
    LAST_TIMINGS["reduce_2"] = _time.time() - _t

    return _unshard(shards, cfg, [r["hT"] for r in r2], NCLS, G2)
